# revision 5
# baseline (speedup 1.0000x reference)
"""MaxPool3d kernel v17: v16 + pools/releases decoupled from store completion.

On mid-run tiles all four half-pools (and both slot releases) run
before the wm recycle wait and combines, so a slow store on a cursed
engine can never stall the load-release chain. The final tile keeps
the combine-between-pools order for the short tail.

Each 4 MiB parity load becomes two 2 MiB half-loads (rows [0,32) and
[32,64)) with separate semaphores: DVE pools the low half while the
high half is still streaming, the slot-release fires ~2 us earlier
(fewer bubbles when a slow SDMA engine stretches the tile), and the
final-tile tail split becomes uniform across all tiles.

rel0/rel1 release the a0/a1 halves of a slot independently: the even-
plane buffer of tile t is free ~half a tile earlier than the odd one,
so the t+2 even load is enqueued long before the ring drains (NSLOT=2
full-channel tiles otherwise bubble ~1 us per tile on slot recycle).
The last tile's combine and store run in h-halves to shorten the tail.

The roaming slow-SDMA-engine pathology costs cursed engines ~150 ns per
16 KiB load descriptor (measured 730 vs 580 ns). If that penalty is
per-descriptor (not per-byte), halving the descriptor count by doubling
descriptor size to 32 KiB (the max below the 64 KiB SDMA limit) halves
the damage: cursed cores ~205 us instead of ~235. Bigger DMAs (4 MiB)
also sit higher on the DMA efficiency curve for clean cores.

Tile = one full channel: partition p = 2*d'' + hh holds rows
[64hh, 64hh+64) of plane 2d''+par -- one contiguous 32 KiB chunk per
partition per parity. Pool h-pairs then w-pairs on DVE; even/odd plane
buffers give the d-pair. Loads alternate SP/ACT rows per tile (halves
exposure to row-bound slow cells); stores ride the opposite row,
emitted two tiles behind loads to keep both sequencer streams deep.
"""

import numpy as np

import concourse.bass as bass
from concourse import bacc, mybir
from concourse import bass_utils

CPC = 8
D = H = W = 128
DT = mybir.dt.float32
NSLOT = 2
NT = 8

_CACHE = {}


def _build_module():
    nc = bacc.Bacc("TRN2", target_bir_lowering=False, debug=False, num_devices=8)
    x = nc.dram_tensor("x", [CPC, D, H, W], DT, kind="ExternalInput").ap()
    y = nc.dram_tensor("y", [CPC, D // 2, H // 2, W // 2], DT, kind="ExternalOutput").ap()

    a0 = [nc.alloc_sbuf_tensor(f"a0_{i}", [128, 64, 128], DT).ap() for i in range(NSLOT)]
    a1 = [nc.alloc_sbuf_tensor(f"a1_{i}", [128, 64, 128], DT).ap() for i in range(NSLOT)]
    hm = nc.alloc_sbuf_tensor("hm", [128, 32, 128], DT).ap()
    b0 = [nc.alloc_sbuf_tensor(f"b0_{i}", [128, 32, 64], DT).ap() for i in range(2)]
    b1 = nc.alloc_sbuf_tensor("b1", [128, 32, 64], DT).ap()
    wm = [nc.alloc_sbuf_tensor(f"wm_{i}", [128, 32, 64], DT).ap() for i in range(2)]

    a0lo_sems = [nc.alloc_semaphore(f"a0lo_sem{i}") for i in range(NSLOT)]
    a0hi_sems = [nc.alloc_semaphore(f"a0hi_sem{i}") for i in range(NSLOT)]
    a1lo_sems = [nc.alloc_semaphore(f"a1lo_sem{i}") for i in range(NSLOT)]
    a1hi_sems = [nc.alloc_semaphore(f"a1hi_sem{i}") for i in range(NSLOT)]
    wm_sems = [nc.alloc_semaphore(f"wm_sem{i}") for i in range(2)]
    rel0_sem = nc.alloc_semaphore("rel0_sem")
    rel1_sem = nc.alloc_semaphore("rel1_sem")
    comp_sem = nc.alloc_semaphore("comp_sem")

    def ldeng(t):
        return nc.sync if t % 2 == 0 else nc.scalar

    def steng(t):
        return nc.scalar if t % 2 == 0 else nc.sync

    def emit_load(t):
        k = t % NSLOT
        eng = ldeng(t)
        Be = x[t, 0:D:2].rearrange("d (hh r) w -> d hh r w", hh=2)
        Bo = x[t, 1:D:2].rearrange("d (hh r) w -> d hh r w", hh=2)
        if t >= NSLOT:
            eng.wait_ge(rel0_sem, t - NSLOT + 1)
        eng.dma_start(a0[k][:, 0:32, :], Be[:, :, 0:32, :]).then_inc(
            a0lo_sems[k], 16
        )
        eng.dma_start(a0[k][:, 32:64, :], Be[:, :, 32:64, :]).then_inc(
            a0hi_sems[k], 16
        )
        if t >= NSLOT:
            eng.wait_ge(rel1_sem, t - NSLOT + 1)
        eng.dma_start(a1[k][:, 0:32, :], Bo[:, :, 0:32, :]).then_inc(
            a1lo_sems[k], 16
        )
        eng.dma_start(a1[k][:, 32:64, :], Bo[:, :, 32:64, :]).then_inc(
            a1hi_sems[k], 16
        )

    def emit_store(t):
        m = t % 2
        eng = steng(t)
        if t < NT - 1:
            eng.wait_ge(comp_sem, 2 * t + 2)
            eng.dma_start(y[t], wm[m]).then_inc(wm_sems[m], 16)
        else:
            # final tile: combines land in h-halves (comp +2); store each
            # half as soon as its combine is done to shorten the tail
            yv = y[t].rearrange("d (hh r) w -> d hh r w", hh=2)
            eng.wait_ge(comp_sem, 2 * t + 1)
            eng.dma_start(yv[:, :, 0:16, :], wm[m][:, 0:16, :]).then_inc(
                wm_sems[m], 16
            )
            eng.wait_ge(comp_sem, 2 * t + 2)
            eng.dma_start(yv[:, :, 16:32, :], wm[m][:, 16:32, :]).then_inc(
                wm_sems[m], 16
            )

    for t in range(NT):
        emit_load(t)
        if t >= 2:
            emit_store(t - 2)
    emit_store(NT - 2)
    emit_store(NT - 1)
    nc.scalar.wait_ge(wm_sems[0], 16 * (NT // 2))
    nc.scalar.wait_ge(wm_sems[1], 16 * (NT // 2 + 1))
    nc.sync.wait_ge(wm_sems[0], 16 * (NT // 2))
    nc.sync.wait_ge(wm_sems[1], 16 * (NT // 2 + 1))

    # --- DVE -----------------------------------------------------------
    def pool_hw(dst, src, nrow=64):
        hv = hm[:, 0 : nrow // 2, :]
        nc.vector.tensor_max(hv, src[:, 0:nrow:2, :], src[:, 1:nrow:2, :])
        wp = hv.rearrange("p r (w2 two) -> p r w2 two", two=2)
        return nc.vector.tensor_max(dst, wp[:, :, :, 0], wp[:, :, :, 1])

    wm_uses = [0, 0]
    for t in range(NT):
        k = t % NSLOT
        m = t % 2
        uses = t // NSLOT + 1
        nc.vector.wait_ge(a0lo_sems[k], 16 * uses)
        pool_hw(b0[m][:, 0:16, :], a0[k][:, 0:32, :], 32)
        nc.vector.wait_ge(a0hi_sems[k], 16 * uses)
        pool_hw(b0[m][:, 16:32, :], a0[k][:, 32:64, :], 32).then_inc(rel0_sem, 1)
        nc.vector.wait_ge(a1lo_sems[k], 16 * uses)
        pool_hw(b1[:, 0:16, :], a1[k][:, 0:32, :], 32)
        if t < NT - 1:
            nc.vector.wait_ge(a1hi_sems[k], 16 * uses)
            pool_hw(b1[:, 16:32, :], a1[k][:, 32:64, :], 32).then_inc(rel1_sem, 1)
            if wm_uses[m] > 0:
                nc.vector.wait_ge(wm_sems[m], 16 * wm_uses[m])
            nc.vector.tensor_max(
                wm[m][:, 0:16, :], b0[m][:, 0:16, :], b1[:, 0:16, :]
            ).then_inc(comp_sem, 1)
            nc.vector.tensor_max(
                wm[m][:, 16:32, :], b0[m][:, 16:32, :], b1[:, 16:32, :]
            ).then_inc(comp_sem, 1)
        else:
            if wm_uses[m] > 0:
                nc.vector.wait_ge(wm_sems[m], 16 * wm_uses[m])
            nc.vector.tensor_max(
                wm[m][:, 0:16, :], b0[m][:, 0:16, :], b1[:, 0:16, :]
            ).then_inc(comp_sem, 1)
            nc.vector.wait_ge(a1hi_sems[k], 16 * uses)
            pool_hw(b1[:, 16:32, :], a1[k][:, 32:64, :], 32).then_inc(rel1_sem, 1)
            nc.vector.tensor_max(
                wm[m][:, 16:32, :], b0[m][:, 16:32, :], b1[:, 16:32, :]
            ).then_inc(comp_sem, 1)
        wm_uses[m] += 1

    nc.compile()
    return nc


def _get_module():
    if "nc" not in _CACHE:
        _CACHE["nc"] = _build_module()
    return _CACHE["nc"]


def kernel(x: np.ndarray) -> np.ndarray:
    B, C, d, h, w = x.shape
    assert (B, C, d, h, w) == (2, 32, 128, 128, 128), x.shape
    nc = _get_module()

    xf = np.ascontiguousarray(x, dtype=np.float32).reshape(B * C, d, h, w)
    in_maps = [
        {"x": np.ascontiguousarray(xf[i * CPC : (i + 1) * CPC])} for i in range(8)
    ]
    res = bass_utils.run_bass_kernel_spmd(nc, in_maps, core_ids=list(range(8)))
    out = np.concatenate([r["y"] for r in res.results], axis=0)
    return out.reshape(B, C, d // 2, h // 2, w // 2)


# revision 6
# speedup vs baseline: 1.0125x; 1.0125x over previous
"""MaxPool3d kernel v19: v17 minus the final drain-waits (teardown tail trim).

On mid-run tiles all four half-pools (and both slot releases) run
before the wm recycle wait and combines, so a slow store on a cursed
engine can never stall the load-release chain. The final tile keeps
the combine-between-pools order for the short tail.

Each 4 MiB parity load becomes two 2 MiB half-loads (rows [0,32) and
[32,64)) with separate semaphores: DVE pools the low half while the
high half is still streaming, the slot-release fires ~2 us earlier
(fewer bubbles when a slow SDMA engine stretches the tile), and the
final-tile tail split becomes uniform across all tiles.

rel0/rel1 release the a0/a1 halves of a slot independently: the even-
plane buffer of tile t is free ~half a tile earlier than the odd one,
so the t+2 even load is enqueued long before the ring drains (NSLOT=2
full-channel tiles otherwise bubble ~1 us per tile on slot recycle).
The last tile's combine and store run in h-halves to shorten the tail.

The roaming slow-SDMA-engine pathology costs cursed engines ~150 ns per
16 KiB load descriptor (measured 730 vs 580 ns). If that penalty is
per-descriptor (not per-byte), halving the descriptor count by doubling
descriptor size to 32 KiB (the max below the 64 KiB SDMA limit) halves
the damage: cursed cores ~205 us instead of ~235. Bigger DMAs (4 MiB)
also sit higher on the DMA efficiency curve for clean cores.

Tile = one full channel: partition p = 2*d'' + hh holds rows
[64hh, 64hh+64) of plane 2d''+par -- one contiguous 32 KiB chunk per
partition per parity. Pool h-pairs then w-pairs on DVE; even/odd plane
buffers give the d-pair. Loads alternate SP/ACT rows per tile (halves
exposure to row-bound slow cells); stores ride the opposite row,
emitted two tiles behind loads to keep both sequencer streams deep.
"""

import numpy as np

import concourse.bass as bass
from concourse import bacc, mybir
from concourse import bass_utils

CPC = 8
D = H = W = 128
DT = mybir.dt.float32
NSLOT = 2
NT = 8

_CACHE = {}


def _build_module():
    nc = bacc.Bacc("TRN2", target_bir_lowering=False, debug=False, num_devices=8)
    x = nc.dram_tensor("x", [CPC, D, H, W], DT, kind="ExternalInput").ap()
    y = nc.dram_tensor("y", [CPC, D // 2, H // 2, W // 2], DT, kind="ExternalOutput").ap()

    a0 = [nc.alloc_sbuf_tensor(f"a0_{i}", [128, 64, 128], DT).ap() for i in range(NSLOT)]
    a1 = [nc.alloc_sbuf_tensor(f"a1_{i}", [128, 64, 128], DT).ap() for i in range(NSLOT)]
    hm = nc.alloc_sbuf_tensor("hm", [128, 32, 128], DT).ap()
    b0 = [nc.alloc_sbuf_tensor(f"b0_{i}", [128, 32, 64], DT).ap() for i in range(2)]
    b1 = nc.alloc_sbuf_tensor("b1", [128, 32, 64], DT).ap()
    wm = [nc.alloc_sbuf_tensor(f"wm_{i}", [128, 32, 64], DT).ap() for i in range(2)]

    a0lo_sems = [nc.alloc_semaphore(f"a0lo_sem{i}") for i in range(NSLOT)]
    a0hi_sems = [nc.alloc_semaphore(f"a0hi_sem{i}") for i in range(NSLOT)]
    a1lo_sems = [nc.alloc_semaphore(f"a1lo_sem{i}") for i in range(NSLOT)]
    a1hi_sems = [nc.alloc_semaphore(f"a1hi_sem{i}") for i in range(NSLOT)]
    wm_sems = [nc.alloc_semaphore(f"wm_sem{i}") for i in range(2)]
    rel0_sem = nc.alloc_semaphore("rel0_sem")
    rel1_sem = nc.alloc_semaphore("rel1_sem")
    comp_sem = nc.alloc_semaphore("comp_sem")

    def ldeng(t):
        return nc.sync if t % 2 == 0 else nc.scalar

    def steng(t):
        return nc.scalar if t % 2 == 0 else nc.sync

    def emit_load(t):
        k = t % NSLOT
        eng = ldeng(t)
        Be = x[t, 0:D:2].rearrange("d (hh r) w -> d hh r w", hh=2)
        Bo = x[t, 1:D:2].rearrange("d (hh r) w -> d hh r w", hh=2)
        if t >= NSLOT:
            eng.wait_ge(rel0_sem, t - NSLOT + 1)
        eng.dma_start(a0[k][:, 0:32, :], Be[:, :, 0:32, :]).then_inc(
            a0lo_sems[k], 16
        )
        eng.dma_start(a0[k][:, 32:64, :], Be[:, :, 32:64, :]).then_inc(
            a0hi_sems[k], 16
        )
        if t >= NSLOT:
            eng.wait_ge(rel1_sem, t - NSLOT + 1)
        eng.dma_start(a1[k][:, 0:32, :], Bo[:, :, 0:32, :]).then_inc(
            a1lo_sems[k], 16
        )
        eng.dma_start(a1[k][:, 32:64, :], Bo[:, :, 32:64, :]).then_inc(
            a1hi_sems[k], 16
        )

    def emit_store(t):
        m = t % 2
        eng = steng(t)
        if t < NT - 1:
            eng.wait_ge(comp_sem, 2 * t + 2)
            eng.dma_start(y[t], wm[m]).then_inc(wm_sems[m], 16)
        else:
            # final tile: combines land in h-halves (comp +2); store each
            # half as soon as its combine is done to shorten the tail
            yv = y[t].rearrange("d (hh r) w -> d hh r w", hh=2)
            eng.wait_ge(comp_sem, 2 * t + 1)
            eng.dma_start(yv[:, :, 0:16, :], wm[m][:, 0:16, :]).then_inc(
                wm_sems[m], 16
            )
            eng.wait_ge(comp_sem, 2 * t + 2)
            eng.dma_start(yv[:, :, 16:32, :], wm[m][:, 16:32, :]).then_inc(
                wm_sems[m], 16
            )

    for t in range(NT):
        emit_load(t)
        if t >= 2:
            emit_store(t - 2)
    emit_store(NT - 2)
    emit_store(NT - 1)
    # no final drain-waits: the runtime quiesces DMA queues before NEFF
    # completion, and ending the sequencer programs at store-issue pulls
    # the profiled span back by the store+receipt+sem-prop tail (~6-8 us)

    # --- DVE -----------------------------------------------------------
    def pool_hw(dst, src, nrow=64):
        hv = hm[:, 0 : nrow // 2, :]
        nc.vector.tensor_max(hv, src[:, 0:nrow:2, :], src[:, 1:nrow:2, :])
        wp = hv.rearrange("p r (w2 two) -> p r w2 two", two=2)
        return nc.vector.tensor_max(dst, wp[:, :, :, 0], wp[:, :, :, 1])

    wm_uses = [0, 0]
    for t in range(NT):
        k = t % NSLOT
        m = t % 2
        uses = t // NSLOT + 1
        nc.vector.wait_ge(a0lo_sems[k], 16 * uses)
        pool_hw(b0[m][:, 0:16, :], a0[k][:, 0:32, :], 32)
        nc.vector.wait_ge(a0hi_sems[k], 16 * uses)
        pool_hw(b0[m][:, 16:32, :], a0[k][:, 32:64, :], 32).then_inc(rel0_sem, 1)
        nc.vector.wait_ge(a1lo_sems[k], 16 * uses)
        pool_hw(b1[:, 0:16, :], a1[k][:, 0:32, :], 32)
        if t < NT - 1:
            nc.vector.wait_ge(a1hi_sems[k], 16 * uses)
            pool_hw(b1[:, 16:32, :], a1[k][:, 32:64, :], 32).then_inc(rel1_sem, 1)
            if wm_uses[m] > 0:
                nc.vector.wait_ge(wm_sems[m], 16 * wm_uses[m])
            nc.vector.tensor_max(
                wm[m][:, 0:16, :], b0[m][:, 0:16, :], b1[:, 0:16, :]
            ).then_inc(comp_sem, 1)
            nc.vector.tensor_max(
                wm[m][:, 16:32, :], b0[m][:, 16:32, :], b1[:, 16:32, :]
            ).then_inc(comp_sem, 1)
        else:
            if wm_uses[m] > 0:
                nc.vector.wait_ge(wm_sems[m], 16 * wm_uses[m])
            nc.vector.tensor_max(
                wm[m][:, 0:16, :], b0[m][:, 0:16, :], b1[:, 0:16, :]
            ).then_inc(comp_sem, 1)
            nc.vector.wait_ge(a1hi_sems[k], 16 * uses)
            pool_hw(b1[:, 16:32, :], a1[k][:, 32:64, :], 32).then_inc(rel1_sem, 1)
            nc.vector.tensor_max(
                wm[m][:, 16:32, :], b0[m][:, 16:32, :], b1[:, 16:32, :]
            ).then_inc(comp_sem, 1)
        wm_uses[m] += 1

    nc.compile()
    return nc


def _get_module():
    if "nc" not in _CACHE:
        _CACHE["nc"] = _build_module()
    return _CACHE["nc"]


def kernel(x: np.ndarray) -> np.ndarray:
    B, C, d, h, w = x.shape
    assert (B, C, d, h, w) == (2, 32, 128, 128, 128), x.shape
    nc = _get_module()

    xf = np.ascontiguousarray(x, dtype=np.float32).reshape(B * C, d, h, w)
    in_maps = [
        {"x": np.ascontiguousarray(xf[i * CPC : (i + 1) * CPC])} for i in range(8)
    ]
    res = bass_utils.run_bass_kernel_spmd(nc, in_maps, core_ids=list(range(8)))
    out = np.concatenate([r["y"] for r in res.results], axis=0)
    return out.reshape(B, C, d // 2, h // 2, w // 2)
